# revision 1
# baseline (speedup 1.0000x reference)
"""EvaAttention Trainium2 Bass kernel.

Data-parallel over batch across 8 cores (4 batches/core), weights
replicated, no collectives. Rewrite of the 878us baseline; key changes:

  - Mixed precision: bf16 for x, qkv weights, attention weights E and V
    tiles (~2e-3 error contribution each); fp32/fp32r for q/k, the S=qk^T
    matmul, and the whole projection path. bf16 q/k alone would cost
    1.2e-2 of the 2e-2 error budget. Measured max-rel err: 6.6e-3.
  - All fp32r matmuls use free-dim splits >= 256 (290/290, 288/288,
    384/384) so they stream at 1 cycle/row; the baseline's 512+66 splits
    paid 4 cycles/row on every 66-wide tail (~27% of its PE time).
  - PSUM tiles are [128, 2, 512] bank pairs; the two matmul halves target
    separate banks and the PSUM->SBUF evacuation (ACT exp / bias copy)
    reads both banks in ONE instruction - the ACT engine charges a ~370ns
    fixed access cost per instruction, so halving the instruction count
    saves ~35us/core.
  - The attention phase is ACT-bound (exp 668ns/[128,580] chunk vs 240ns
    of S matmul): the previous batch's projection and the current batch's
    V matmuls are woven into the attention head loop so the in-order PE
    queue has ready work while exp paces S/PV.
  - S matmuls for a head pair are emitted interleaved: even head's kk on
    SBUF partitions 0:64 (PE row groups 0-1), odd head's on 64:128
    (groups 2-3); auto tile_position runs them concurrently on the PE's
    16 32x32 subarrays.
  - Projection path (oT, proj_w) stays fp32r: with bf16 oT the P-phase
    LDWEIGHTS takes the fast-weight-load path (FWL, 4-XBUS 32-bit reads)
    on a tile whose odd-head half was just DMA'd; on HW this corrupted
    every second token in 8-partition bands (deterministic, schedule-
    dependent, invisible in CoreSim). fp32 weights never use FWL and the
    4-byte om DMA matches the baseline's proven path.
  - Renorm broadcast via DRAM round trip: gpsimd partition_broadcast
    corrupted back-to-back invocations on HW, and SBUF sources cannot
    have the stride-0 partition APs a direct SBUF->SBUF broadcast needs.
"""

import os
import numpy as np

import concourse.bacc as bacc
import concourse.tile as tile
import concourse.mybir as mybir
from concourse.bass_utils import run_bass_kernel_spmd

B, N, C = 32, 577, 768
H, D = 12, 64
NPT = 1
N_CORES = 8
NB = B // N_CORES          # batches per core
TT = NB * N                # tokens per core (2308)
TP = TT + 4                # padded (2312)
SCALE = D ** -0.5

K_REP = int(os.environ.get("BASS_K_REP", "1"))
RENORM = os.environ.get("BASS_RENORM", "dram")  # dram | gpsimd

_f32 = mybir.dt.float32
_f32r = mybir.dt.float32r
_bf16 = mybir.dt.bfloat16
_NP_BF16 = mybir.dt.np(_bf16)

def preprocess(x, rope, qkv_w, q_bias, v_bias, proj_w, proj_b):
    """Host-side prep: transposes, head-d permutation, rope tables."""
    perm = np.concatenate([np.arange(0, D, 2), np.arange(1, D, 2)])
    rows = np.arange(3 * C)
    for region in (0, 1):  # q, k head-d reorder; v untouched
        for h in range(H):
            base = region * C + h * D
            rows[base : base + D] = base + perm
    qkv_w_p = np.asarray(qkv_w, np.float32)[rows]
    qkvb_flat = np.concatenate(
        [np.asarray(q_bias, np.float32), np.zeros(C, np.float32),
         np.asarray(v_bias, np.float32)])[rows]
    qkvb = np.ascontiguousarray(qkvb_flat.reshape(18, 128).T)  # [128, 18]

    wT = np.ascontiguousarray(qkv_w_p.T).astype(_NP_BF16)      # [768, 2304]
    # proj weights stay f32: the projection runs as fp32r so its LDWEIGHTS
    # path never uses FWL (see module docstring)
    pwT = np.ascontiguousarray(np.asarray(proj_w, np.float32).T)

    rope = np.asarray(rope, np.float32)
    sinT = np.ascontiguousarray(rope[:, :D].T[perm])           # [64, 576]
    cosT = np.ascontiguousarray(rope[:, D:].T[perm])
    cosT2 = np.concatenate([cosT, cosT], 0).astype(np.float32) # [128, 576]
    sinT2 = np.concatenate([sinT, sinT], 0).astype(np.float32)

    # rot() after the evens/odds permutation: per 64-block partition swap
    # out[0:32] = -in[32:64]; out[32:64] = +in[0:32]
    rotm = np.zeros((128, 128), np.float32)
    for blk in range(2):
        o = blk * 64
        for j in range(32):
            rotm[o + 32 + j, o + j] = -1.0
            rotm[o + j, o + 32 + j] = 1.0

    x = np.asarray(x, np.float32)
    xTs = []
    for core in range(N_CORES):
        xc = x[core * NB : (core + 1) * NB]                    # [NB, 577, 768]
        xT = np.zeros((C, TP), np.float32)
        xT[:, :TT] = xc.transpose(2, 0, 1).reshape(C, TT)
        xTs.append(xT.astype(_NP_BF16))

    vb = np.asarray(v_bias, np.float32)
    pb = np.asarray(proj_b, np.float32)
    return xTs, dict(wT=wT, pwT=pwT, qkvb=qkvb, vb=vb, pb=pb,
                     cosT2=cosT2, sinT2=sinT2, rotm=rotm)


def build(mode=None, k_rep=K_REP):
    nc = bacc.Bacc("TRN2", target_bir_lowering=False, debug=False,
                   num_devices=N_CORES)

    d_xT = nc.dram_tensor("xT", [C, TP], _bf16, kind="ExternalInput").ap()
    d_wT = nc.dram_tensor("wT", [C, 3 * C], _bf16, kind="ExternalInput").ap()
    d_pwT = nc.dram_tensor("pwT", [C, C], _f32r, kind="ExternalInput").ap()
    d_qkvb = nc.dram_tensor("qkvb", [128, 18], _f32, kind="ExternalInput").ap()
    d_vb = nc.dram_tensor("vb", [C], _f32, kind="ExternalInput").ap()
    d_pb = nc.dram_tensor("pb", [C], _f32, kind="ExternalInput").ap()
    d_cos = nc.dram_tensor("cosT2", [128, N - 1], _f32, kind="ExternalInput").ap()
    d_sin = nc.dram_tensor("sinT2", [128, N - 1], _f32, kind="ExternalInput").ap()
    d_rotm = nc.dram_tensor("rotm", [128, 128], _f32r, kind="ExternalInput").ap()
    d_out = nc.dram_tensor("out", [TT, C], _f32, kind="ExternalOutput").ap()

    Id = mybir.ActivationFunctionType.Identity
    Exp = mybir.ActivationFunctionType.Exp

    import concourse.bass as bass_mod

    def _row_bc(ap, parts):
        return bass_mod.AP(tensor=ap.tensor, offset=ap.offset,
                           ap=[[0, parts]] + list(ap.ap))

    with tile.TileContext(nc) as tc:
        with tc.tile_pool(name="main", bufs=1) as pool, \
             tc.tile_pool(name="ps", bufs=1, space="PSUM") as pspool, \
             tc.tile_pool(name="dr", bufs=1, space="DRAM") as drpool:

            # ---- resident constants ----
            wT_sb = pool.tile([128, 6, 3 * C], _bf16, tag="wT")
            pwT_sb = pool.tile([128, 6, C], _f32r, tag="pwT")
            for c in range(6):
                nc.sync.dma_start(out=wT_sb[:, c, :], in_=d_wT[c * 128:(c + 1) * 128, :])
                nc.sync.dma_start(out=pwT_sb[:, c, :], in_=d_pwT[c * 128:(c + 1) * 128, :])
            qkvb_sb = pool.tile([128, 18], _f32, tag="qkvb")
            nc.sync.dma_start(out=qkvb_sb, in_=d_qkvb)
            vb_bc = pool.tile([128, C], _f32, tag="vb")
            nc.sync.dma_start(out=vb_bc, in_=_row_bc(d_vb, 128))
            pb_bc = pool.tile([128, C], _f32, tag="pb")
            nc.sync.dma_start(out=pb_bc, in_=_row_bc(d_pb, 128))
            cos_sb = pool.tile([128, N - 1], _f32, tag="cos")
            nc.sync.dma_start(out=cos_sb, in_=d_cos)
            sin_sb = pool.tile([128, N - 1], _f32, tag="sin")
            nc.sync.dma_start(out=sin_sb, in_=d_sin)
            rotm_sb = pool.tile([128, 128], _f32r, tag="rotm")
            nc.sync.dma_start(out=rotm_sb, in_=d_rotm)

            def load_xT(b):
                t0 = b * N
                xT_b = pool.tile([128, 6, 580], _bf16, tag="xTb", bufs=2)
                for c in range(6):
                    nc.sync.dma_start(out=xT_b[:, c, :],
                                      in_=d_xT[c * 128:(c + 1) * 128,
                                               t0:t0 + 580])
                return xT_b

            def emit_A(b, qk, xT_b, ots=range(12)):
                for ot in ots:
                    psA = pspool.tile([128, 2, 512], _f32, tag="ps2", bufs=2)
                    for c in range(6):
                        lhsT = wT_sb[:, c, ot * 128:(ot + 1) * 128]
                        nc.tensor.matmul(psA[:, 0, 0:290], lhsT,
                                         xT_b[:, c, 0:290],
                                         start=c == 0, stop=c == 5)
                        nc.tensor.matmul(psA[:, 1, 0:290], lhsT,
                                         xT_b[:, c, 290:580],
                                         start=c == 0, stop=c == 5)
                    nc.scalar.activation(
                        qk[:, ot, :].rearrange("p (a b) -> p a b", b=290),
                        psA[:, :, 0:290], Id, bias=qkvb_sb[:, ot:ot + 1])

            def emit_R(ot, qk):
                psR = pspool.tile([128, 2, 512], _f32, tag="ps2", bufs=2)
                nc.tensor.matmul(psR[:, 0, 0:288], rotm_sb,
                                 qk[:, ot, 1:289], start=True, stop=True)
                nc.tensor.matmul(psR[:, 1, 0:288], rotm_sb,
                                 qk[:, ot, 289:577], start=True, stop=True)
                rt = pool.tile([128, 2, 288], _f32, tag="rt", bufs=1)
                nc.vector.tensor_mul(
                    rt, psR[:, :, 0:288],
                    sin_sb.rearrange("p (a b) -> p a b", b=288))
                nc.vector.tensor_mul(qk[:, ot, 1:577], qk[:, ot, 1:577], cos_sb)
                nc.vector.tensor_add(qk[:, ot, 1:577], qk[:, ot, 1:577],
                                     rt.rearrange("p a b -> p (a b)"))

            def emit_B(b, tt, v, xT_b):
                P = min(128, N - tt * 128)
                PM = P + (P % 2)
                psB = pspool.tile([128, 2, 512], _f32, tag="ps2", bufs=2)
                for c in range(6):
                    lhsT = xT_b[:, c, tt * 128:tt * 128 + PM]
                    nc.tensor.matmul(psB[:PM, 0, 0:384], lhsT,
                                     wT_sb[:, c, 1536:1920],
                                     start=c == 0, stop=c == 5)
                    nc.tensor.matmul(psB[:PM, 1, 0:384], lhsT,
                                     wT_sb[:, c, 1920:2304],
                                     start=c == 0, stop=c == 5)
                nc.vector.tensor_add(
                    v[0:P, tt, :, 0:64].rearrange("p (a h) d -> p a h d", a=2),
                    psB[:P, :, 0:384].rearrange("p a (h d) -> p a h d", d=64),
                    vb_bc[0:P, :].rearrange("p (a h d) -> p a h d", a=2, d=64))

            def emit_S_pair(hp, qk, Es):
                ot = hp
                for kc in range(5):
                    KP = min(128, N + 1 - kc * 128)
                    for s in range(2):  # even head rows 0:64, odd rows 64:128
                        hb = s * 64
                        psS = pspool.tile([128, 2, 512], _f32, tag="ps2",
                                          bufs=2)
                        kk = qk[hb:hb + 64, 6 + ot, kc * 128:kc * 128 + KP]
                        nc.tensor.matmul(psS[:KP, 0, 0:290], kk,
                                         qk[hb:hb + 64, ot, 0:290],
                                         start=True, stop=True)
                        nc.tensor.matmul(psS[:KP, 1, 0:290], kk,
                                         qk[hb:hb + 64, ot, 290:580],
                                         start=True, stop=True)
                        nc.scalar.activation(Es[s][0:KP, kc],
                                             psS[0:KP, :, 0:290],
                                             Exp, scale=SCALE)

            def emit_PV_renorm_pair(hp, v, Es, oT):
                ot = hp
                for s in range(2):
                    h = 2 * hp + s
                    psO = pspool.tile([128, 2, 512], _f32, tag="psO", bufs=2)
                    for kc in range(5):
                        KP = min(128, N + 1 - kc * 128)
                        vv = v[0:KP, kc, h, :]
                        nc.tensor.matmul(psO[:66, 0, 0:290], vv,
                                         Es[s][0:KP, kc, 0], start=kc == 0,
                                         stop=kc == 4)
                        nc.tensor.matmul(psO[:66, 1, 0:290], vv,
                                         Es[s][0:KP, kc, 1], start=kc == 0,
                                         stop=kc == 4)
                    r = pool.tile([128, 580], _f32, tag="r", bufs=1)
                    nc.vector.reciprocal(
                        r[64:65, :].rearrange("p (a b) -> p a b", b=290),
                        psO[64:65, :, 0:290])
                    rb = pool.tile([64, 580], _f32, tag="rb", bufs=1)
                    if RENORM == "gpsimd":
                        r0 = pool.tile([1, 580], _f32, tag="r0", bufs=2)
                        nc.sync.dma_start(out=r0, in_=r[64:65, :])
                        nc.gpsimd.partition_broadcast(rb, r0)
                    else:
                        rrow = drpool.tile([1, 580], _f32, tag="rrow", bufs=4)
                        nc.sync.dma_start(out=rrow, in_=r[64:65, :])
                        nc.sync.dma_start(out=rb, in_=_row_bc(rrow[0, :], 64))
                    rbv = rb.rearrange("p (a b) -> p a b", b=290)
                    if s == 0:
                        nc.vector.tensor_mul(
                            oT[0:64, ot, :].rearrange("p (a b) -> p a b",
                                                      b=290),
                            psO[0:64, :, 0:290], rbv)
                    else:
                        om = pool.tile([64, 580], _f32r, tag="om", bufs=2)
                        nc.vector.tensor_mul(
                            om.rearrange("p (a b) -> p a b", b=290),
                            psO[0:64, :, 0:290], rbv)
                        nc.sync.dma_start(out=oT[64:128, ot, :], in_=om)

            def emit_P(b, tt, oT):
                t0 = b * N
                P = min(128, N - tt * 128)
                PM = P + (P % 2)
                psP = pspool.tile([128, 2, 512], _f32, tag="ps2", bufs=2)
                for c in range(6):
                    lhsT = oT[:, c, tt * 128:tt * 128 + PM]
                    nc.tensor.matmul(psP[:PM, 0, 0:384], lhsT,
                                     pwT_sb[:, c, 0:384],
                                     start=c == 0, stop=c == 5)
                    nc.tensor.matmul(psP[:PM, 1, 0:384], lhsT,
                                     pwT_sb[:, c, 384:768],
                                     start=c == 0, stop=c == 5)
                y = pool.tile([128, C], _f32, tag="y", bufs=2)
                nc.vector.tensor_add(
                    y[:P].rearrange("p (a b) -> p a b", b=384),
                    psP[:P, :, 0:384],
                    pb_bc[:P].rearrange("p (a b) -> p a b", b=384))
                nc.sync.dma_start(
                    out=d_out[t0 + tt * 128:t0 + tt * 128 + P, :], in_=y[:P])

            def body():
                prev = None  # (b, oT) of previous batch awaiting projection
                qk_next = None
                xT_next = None
                for b in range(NB):
                    if qk_next is None:
                        qk = pool.tile([128, 12, 580], _f32r, tag="qk", bufs=2)
                        xT_b = load_xT(b)
                        emit_A(b, qk, xT_b)
                    else:
                        # A(b) already emitted during batch b-1's attention
                        qk, xT_b = qk_next, xT_next
                    v = pool.tile([128, 5, H, 66], _bf16, tag="v", bufs=2)
                    oT = pool.tile([128, 6, 580], _f32r, tag="oT", bufs=2)
                    nc.vector.memset(v[:, :, :, 64:66], 1.0)
                    nc.vector.memset(v[64:128, 4, :, :], 0.0)
                    nc.vector.memset(v[64:65, 4, :, 64:66], 1.0)
                    for ot in range(12):
                        emit_R(ot, qk)

                    qk_next = xT_next = None
                    Es_prev = None
                    for hp in range(6):
                        E0 = pool.tile([128, 5, 2, 290], _bf16, tag="E0",
                                       bufs=2)
                        E1 = pool.tile([128, 5, 2, 290], _bf16, tag="E1",
                                       bufs=2)
                        Es = (E0, E1)
                        emit_S_pair(hp, qk, Es)
                        # weave into the exp-paced attention stretch: B of
                        # this batch, P of the previous, A of the next
                        if hp == 0:
                            for tt in range(5):
                                emit_B(b, tt, v, xT_b)
                        if hp > 0:
                            emit_PV_renorm_pair(hp - 1, v, Es_prev, oT)
                        if prev is not None and hp > 0:
                            emit_P(prev[0], hp - 1, prev[1])
                        if b + 1 < NB and hp >= 2:
                            if hp == 2:
                                qk_next = pool.tile([128, 12, 580], _f32r,
                                                    tag="qk", bufs=2)
                                xT_next = load_xT(b + 1)
                            emit_A(b + 1, qk_next, xT_next,
                                   ots=range(3 * (hp - 2), 3 * (hp - 1)))
                        Es_prev = Es
                    emit_PV_renorm_pair(5, v, Es_prev, oT)
                    prev = (b, oT)
                # drain: projection of the last batch
                for tt in range(5):
                    emit_P(prev[0], tt, prev[1])

            if k_rep > 1:
                with tc.For_i(0, k_rep, 1):
                    body()
            else:
                body()

    nc.compile()
    return nc


_CACHE = {}


def _get_nc(mode=None, k_rep=K_REP):
    key = (mode, k_rep)
    if key not in _CACHE:
        _CACHE[key] = build(mode, k_rep)
    return _CACHE[key]


def kernel(**inputs) -> np.ndarray:
    xTs, pre = preprocess(**inputs)
    nc = _get_nc()
    shared = {k: pre[k] for k in
              ("wT", "pwT", "qkvb", "vb", "pb", "cosT2", "sinT2", "rotm")}
    in_maps = [dict(shared, xT=xTs[core]) for core in range(N_CORES)]
    res = run_bass_kernel_spmd(nc, in_maps, list(range(N_CORES)))
    out = np.concatenate(
        [res.results[c]["out"].reshape(NB, N, C) for c in range(N_CORES)], axis=0)
    return out

